# revision 1
# baseline (speedup 1.0000x reference)
"""Trainium2 Bass kernel for fused causal GQA attention block.

Reference computation (B=1, S=2048, H=4096, NH=32, NKV=8, HD=128):
    qkv = hs @ w_attn.T; rope(q), rope(k); causal GQA attention;
    out @ w_proj.T

Sharding (8 cores, tensor parallel): core i owns kv-group i = rows
[i*768, (i+1)*768) of w_attn (4 q heads + 1 k + 1 v head) and rows
[i*512, (i+1)*512) of w_proj.  Each core computes its 4 heads of
attention output transposed (feature-major); a seq-chunked AllGather
(4 x [512, 512] per core -> [4096, 512]) assembles attnT on every core
pipelined against the next attention block, and each core computes its
512 output columns of the final projection per seq chunk.

All heavy matmuls run in float32r (tf32, full-rate fp32 PE path).
DMA triggers are spread across engine queues (sync / vector / scalar),
with gpsimd reserved for the collectives, and PSUM->SBUF copies run on
DVE to keep ACT free for the softmax exp.
"""

import sys

sys.path.insert(0, "/opt/trn_rl_repo")

import numpy as np

import concourse.bass as bass
import concourse.tile as tile
from concourse import bacc, mybir
from concourse.bass_utils import run_bass_kernel_spmd

F32 = mybir.dt.float32
F32R = mybir.dt.float32r

B, S, H = 1, 2048, 4096
NH, NKV, HD = 32, 8, 128
GROUP = NH // NKV  # 4
SCALE = 0.08838834764831845
NCORES = 8

M_SHARD = (GROUP + 2) * HD  # 768 rows of w_attn per core
P_SHARD = H // NCORES  # 512 rows of w_proj per core

KC = H // 128  # 32 contraction chunks of the model dim
NB = S // 512  # 4 seq blocks of 512
MT = M_SHARD // 128  # 6 row tiles of qkv_t
ST = S // 128  # 16 seq tiles of 128


def build_module() -> bass.Bass:
    nc = bacc.Bacc(
        "TRN2",
        target_bir_lowering=False,
        debug=False,
        num_devices=NCORES,
    )

    hs_t = nc.dram_tensor("hs_t", [H, S], F32R, kind="ExternalInput")
    wa_t = nc.dram_tensor("wa_t", [H, M_SHARD], F32R, kind="ExternalInput")
    wp_t = nc.dram_tensor("wp_t", [H, P_SHARD], F32R, kind="ExternalInput")
    cos_t = nc.dram_tensor("cos_t", [HD, S], F32R, kind="ExternalInput")
    sin_t = nc.dram_tensor("sin_t", [HD, S], F32R, kind="ExternalInput")
    rot_t = nc.dram_tensor("rot_t", [HD, HD], F32R, kind="ExternalInput")
    mask_sl = nc.dram_tensor("mask_sl", [128, 1024], F32R, kind="ExternalInput")
    ones_in = nc.dram_tensor("ones_in", [128, 128], F32R, kind="ExternalInput")
    ident_in = nc.dram_tensor("ident_in", [128, 128], F32R, kind="ExternalInput")
    y_out = nc.dram_tensor("y", [S, P_SHARD], F32, kind="ExternalOutput")

    # per-seq-chunk collective buffers
    ag_ins = [
        nc.dram_tensor(f"ag_in{i}", [GROUP * HD, 512], F32R, kind="Internal")
        for i in range(NB)
    ]
    ag_outs = [
        nc.dram_tensor(
            f"ag_out{i}", [H, 512], F32R, kind="Internal", addr_space="Shared"
        )
        for i in range(NB)
    ]

    # DRAM views with 128-partition tiling of the contraction axis
    hs_v = hs_t[:].rearrange("(ko p) n -> p ko n", p=128)  # [128, 32, 2048]
    wa_v = wa_t[:].rearrange("(ko p) m -> p ko m", p=128)  # [128, 32, 768]
    wp_v = wp_t[:].rearrange("(ko p) m -> p ko m", p=128)  # [128, 32, 512]
    ag_vs = [a[:].rearrange("(ko p) n -> p ko n", p=128) for a in ag_outs]

    with tile.TileContext(nc) as tc:
        # ---------- persistent pools ----------
        qkv_pool = tc.alloc_tile_pool(name="qkv", bufs=1)
        const_pool = tc.alloc_tile_pool(name="consts", bufs=1)

        qkv_sb = qkv_pool.tile([128, MT, S], F32R)  # 48KB/part

        ones_sb = const_pool.tile([128, 128], F32R)
        ident_sb = const_pool.tile([128, 128], F32R)
        rot_sb = const_pool.tile([128, HD], F32R)
        mask_sb = const_pool.tile([128, 1024], F32R)
        nc.scalar.dma_start(out=ones_sb, in_=ones_in[:])
        nc.scalar.dma_start(out=ident_sb, in_=ident_in[:])
        nc.scalar.dma_start(out=rot_sb, in_=rot_t[:])
        nc.scalar.dma_start(out=mask_sb, in_=mask_sl[:])

        # ---------- phase A: qkv_t = wa_shard @ hs.T ----------
        with (
            tc.tile_pool(name="wa", bufs=1) as wa_pool,
            tc.tile_pool(name="hs", bufs=2) as hs_pool,
            tc.tile_pool(name="psA", bufs=1, space="PSUM") as psA,
        ):
            wa_sb = wa_pool.tile([128, KC, M_SHARD], F32R)  # 96KB/part
            # wa on the vector queue so it overlaps the hs stream on sync
            for si, kk in enumerate(range(0, KC, 8)):
                eng = nc.scalar if si % 2 == 0 else nc.sync
                eng.dma_start(
                    out=wa_sb[:, kk : kk + 8, :], in_=wa_v[:, kk : kk + 8, :]
                )
            KSLAB = 8
            for nb in range(NB):
                psums = [
                    psA.tile([128, 512], F32, tag=f"ps{m}", name=f"psA{m}")
                    for m in range(MT)
                ]
                for ks in range(0, KC, KSLAB):
                    hs_slab = hs_pool.tile(
                        [128, KSLAB, 512], F32R, name="hs_slab"
                    )  # 16KB/part
                    eng = nc.sync if (nb * 4 + ks // KSLAB) % 2 == 0 else nc.scalar
                    eng.dma_start(
                        out=hs_slab,
                        in_=hs_v[:, ks : ks + KSLAB, nb * 512 : (nb + 1) * 512],
                    )
                    for k in range(ks, ks + KSLAB):
                        for m in range(MT):
                            nc.tensor.matmul(
                                psums[m],
                                lhsT=wa_sb[:, k, m * 128 : (m + 1) * 128],
                                rhs=hs_slab[:, k - ks, :],
                                start=(k == 0),
                                stop=(k == KC - 1),
                            )
                for m in range(MT):
                    nc.vector.tensor_copy(
                        out=qkv_sb[:, m, nb * 512 : (nb + 1) * 512], in_=psums[m]
                    )

        # ---------- phase B+C: rope, attention, chunked AG, c_proj ----------
        with (
            tc.tile_pool(name="wp", bufs=1) as wp_pool,
            tc.tile_pool(name="vnat", bufs=1) as vnat_pool,
        ):
            # w_proj shard: DMA overlaps rope/attention compute
            wp_sb = wp_pool.tile([128, KC, P_SHARD], F32R)  # 64KB/part
            for kk in range(0, KC, 8):
                nc.scalar.dma_start(
                    out=wp_sb[:, kk : kk + 8, :], in_=wp_v[:, kk : kk + 8, :]
                )

            v_nat = vnat_pool.tile([128, ST, HD], F32R)  # 8KB/part

            with (
                tc.tile_pool(name="rope", bufs=2) as rope_pool,
                tc.tile_pool(name="psR", bufs=2, space="PSUM") as psR,
            ):
                cos_sb = rope_pool.tile([128, S], F32R, tag="cos")
                sin_sb = rope_pool.tile([128, S], F32R, tag="sin")
                nc.sync.dma_start(out=cos_sb, in_=cos_t[:])
                nc.sync.dma_start(out=sin_sb, in_=sin_t[:])

                # rope on q0..q3 and k (tiles 0..4 of qkv_sb), in place
                for t in range(GROUP + 1):
                    x = qkv_sb[:, t, :]
                    for blk in range(NB):
                        rp = psR.tile([128, 512], F32, name="rp")
                        nc.tensor.matmul(
                            rp,
                            lhsT=rot_sb[:],
                            rhs=x[:, blk * 512 : (blk + 1) * 512],
                            start=True,
                            stop=True,
                        )
                        rs = rope_pool.tile([128, 512], F32R, tag="rs", name="rs")
                        nc.vector.tensor_mul(
                            rs, rp, sin_sb[:, blk * 512 : (blk + 1) * 512]
                        )
                        nc.vector.tensor_mul(
                            x[:, blk * 512 : (blk + 1) * 512],
                            x[:, blk * 512 : (blk + 1) * 512],
                            cos_sb[:, blk * 512 : (blk + 1) * 512],
                        )
                        nc.vector.tensor_add(
                            x[:, blk * 512 : (blk + 1) * 512],
                            x[:, blk * 512 : (blk + 1) * 512],
                            rs,
                        )

                # v natural layout: 16 PE transposes of vT chunks
                for j in range(ST):
                    tp = psR.tile([128, 128], F32R, tag="tp", name="tp")
                    nc.tensor.transpose(
                        tp,
                        qkv_sb[:, GROUP + 1, j * 128 : (j + 1) * 128],
                        ident_sb[:],
                    )
                    nc.vector.tensor_copy(out=v_nat[:, j, :], in_=tp)

            with (
                tc.tile_pool(name="pt", bufs=3) as pt_pool,
                tc.tile_pool(name="attn", bufs=2) as attn_pool,
                tc.tile_pool(name="agl", bufs=2) as agl_pool,
                tc.tile_pool(name="ysb", bufs=2) as y_pool,
                tc.tile_pool(name="psS", bufs=2, space="PSUM") as psS,
                tc.tile_pool(name="psL", bufs=2, space="PSUM") as psL,
                tc.tile_pool(name="psO", bufs=2, space="PSUM") as psO,
                tc.tile_pool(name="psC", bufs=2, space="PSUM") as psC,
            ):
                kT = qkv_sb[:, GROUP, :]
                for iq in range(NB):
                    njb = 4 * iq + 4
                    for h in range(GROUP):
                        qs = qkv_sb[:, h, iq * 512 : (iq + 1) * 512]
                        l_ps = psL.tile([128, 512], F32, name="l_ps")
                        o_ps = psO.tile([128, 512], F32, name="o_ps")
                        for j in range(njb):
                            st = psS.tile([128, 512], F32, name="st")
                            nc.tensor.matmul(
                                st,
                                lhsT=kT[:, j * 128 : (j + 1) * 128],
                                rhs=qs,
                                start=True,
                                stop=True,
                            )
                            pt = pt_pool.tile([128, 512], F32R, name="pt")
                            nc.scalar.activation(
                                out=pt,
                                in_=st,
                                func=mybir.ActivationFunctionType.Exp,
                                scale=SCALE,
                            )
                            off = j * 128 - iq * 512
                            if off >= 0:  # diagonal chunk: causal 0/1 mask
                                nc.vector.tensor_mul(
                                    pt, pt, mask_sb[:, 512 - off : 1024 - off]
                                )
                            nc.tensor.matmul(
                                l_ps,
                                lhsT=ones_sb[:],
                                rhs=pt[:],
                                start=(j == 0),
                                stop=(j == njb - 1),
                            )
                            nc.tensor.matmul(
                                o_ps,
                                lhsT=v_nat[:, j, :],
                                rhs=pt[:],
                                start=(j == 0),
                                stop=(j == njb - 1),
                            )
                        linv = attn_pool.tile(
                            [128, 512], F32, tag="linv", name="linv"
                        )
                        nc.vector.reciprocal(linv, l_ps)
                        at = attn_pool.tile([128, 512], F32R, tag="at", name="at")
                        nc.vector.tensor_mul(at, o_ps, linv)
                        nc.sync.dma_start(
                            out=ag_ins[iq][h * 128 : (h + 1) * 128, :], in_=at
                        )

                    # ---- seq-chunked AllGather (overlaps next iq's compute)
                    nc.gpsimd.collective_compute(
                        "AllGather",
                        mybir.AluOpType.bypass,
                        replica_groups=[list(range(NCORES))],
                        ins=[ag_ins[iq][:]],
                        outs=[ag_outs[iq][:]],
                    )

                    # ---- c_proj for this seq chunk: 4 row-tiles of 128
                    for sub in range(4):
                        mt = iq * 4 + sub
                        lh = agl_pool.tile([128, KC, 128], F32R, name="lh")
                        nc.sync.dma_start(
                            out=lh,
                            in_=ag_vs[iq][:, :, sub * 128 : (sub + 1) * 128],
                        )
                        yp = psC.tile([128, 512], F32, name="yp")
                        for k in range(KC):
                            nc.tensor.matmul(
                                yp,
                                lhsT=lh[:, k, :],
                                rhs=wp_sb[:, k, :],
                                start=(k == 0),
                                stop=(k == KC - 1),
                            )
                        ysb = y_pool.tile([128, P_SHARD], F32, name="ysb")
                        nc.vector.tensor_copy(out=ysb, in_=yp)
                        nc.sync.dma_start(
                            out=y_out[mt * 128 : (mt + 1) * 128, :], in_=ysb
                        )

        const_pool.release()
        qkv_pool.release()

    nc.compile()
    return nc


_CACHED = {}


def _get_module():
    if "nc" not in _CACHED:
        _CACHED["nc"] = build_module()
    return _CACHED["nc"]


def make_in_maps(hidden_states, w_attn, w_proj, rope_cos, rope_sin):
    hidden_states = np.asarray(hidden_states, dtype=np.float32)
    w_attn = np.asarray(w_attn, dtype=np.float32)
    w_proj = np.asarray(w_proj, dtype=np.float32)
    rope_cos = np.asarray(rope_cos, dtype=np.float32)
    rope_sin = np.asarray(rope_sin, dtype=np.float32)

    hs_t = np.ascontiguousarray(hidden_states.reshape(S, H).T)
    cos_t = np.ascontiguousarray(rope_cos.T)
    sin_t = np.ascontiguousarray(rope_sin.T)

    # rotate-half as a matmul: rot(x) = R @ x for x in [HD, S] layout,
    # rot_t = R.T so that lhsT.T @ x = R @ x
    rot_t = np.zeros((HD, HD), dtype=np.float32)
    half = HD // 2
    rot_t[half + np.arange(half), np.arange(half)] = -1.0
    rot_t[np.arange(half), half + np.arange(half)] = 1.0

    # causal staircase: mask_sl[r, c] = 1 iff c >= r + 512
    rr, cc = np.meshgrid(np.arange(128), np.arange(1024), indexing="ij")
    mask_sl = (cc >= rr + 512).astype(np.float32)

    ones = np.ones((128, 128), dtype=np.float32)
    ident = np.eye(128, dtype=np.float32)

    in_maps = []
    for i in range(NCORES):
        wa_sh = w_attn[i * M_SHARD : (i + 1) * M_SHARD, :]
        wp_sh = w_proj[i * P_SHARD : (i + 1) * P_SHARD, :]
        in_maps.append(
            {
                "hs_t": hs_t,
                "wa_t": np.ascontiguousarray(wa_sh.T),
                "wp_t": np.ascontiguousarray(wp_sh.T),
                "cos_t": cos_t,
                "sin_t": sin_t,
                "rot_t": rot_t,
                "mask_sl": mask_sl,
                "ones_in": ones,
                "ident_in": ident,
            }
        )
    return in_maps


def kernel(hidden_states, w_attn, w_proj, rope_cos, rope_sin, **_unused):
    nc = _get_module()
    in_maps = make_in_maps(hidden_states, w_attn, w_proj, rope_cos, rope_sin)
    res = run_bass_kernel_spmd(nc, in_maps, core_ids=list(range(NCORES)))

    out = np.empty((S, H), dtype=np.float32)
    for i in range(NCORES):
        out[:, i * P_SHARD : (i + 1) * P_SHARD] = res.results[i]["y"]
    return out.reshape(B, S, H)



# revision 6
# speedup vs baseline: 1.3651x; 1.3651x over previous
"""Trainium2 Bass kernel for fused causal GQA attention block.

Reference computation (B=1, S=2048, H=4096, NH=32, NKV=8, HD=128):
    qkv = hs @ w_attn.T; rope(q), rope(k); causal GQA attention;
    out @ w_proj.T

Sharding (8 cores, tensor parallel): core i owns kv-group i = rows
[i*768, (i+1)*768) of w_attn (4 q heads + 1 k + 1 v head) and rows
[i*512, (i+1)*512) of w_proj.  Each core computes its 4 heads of
attention output transposed (feature-major); a seq-chunked AllGather
assembles attnT on every core, and each core computes its 512 output
columns of the final projection per seq chunk.

v2 design (vs the f32r baseline):
  * all matmul/DMA data in bf16 (host-converted) -- halves HBM and
    collective traffic and the LDWEIGHTS stream; psums stay fp32.
  * RoPE + v-transposes interleaved into phase A per 512-seq block so
    attention starts immediately after the last qkv block.
  * attention runs seq chunks in order iq=3,2,1,0 (largest first) with
    each chunk's AllGather issued right away and its c_proj placed one
    chunk later, so every collective hides under compute.
  * causal mask applied on the PE as a staircase bias-matmul into the
    scores psum (lhsT=identity) -- no DVE hop between exp and l/o.
  * scores software-pipelined 3 deep over key tiles; exp on ACT.
  * softmax reciprocal via reciprocal_approx_fast (~5x faster).
"""

import sys

sys.path.insert(0, "/opt/trn_rl_repo")

import ml_dtypes
import numpy as np

import concourse.bass as bass
import concourse.tile as tile
from concourse import bacc, mybir
from concourse.bass_utils import run_bass_kernel_spmd

F32 = mybir.dt.float32
BF16 = mybir.dt.bfloat16

B, S, H = 1, 2048, 4096
NH, NKV, HD = 32, 8, 128
GROUP = NH // NKV  # 4
SCALE = 0.08838834764831845
NCORES = 8

M_SHARD = (GROUP + 2) * HD  # 768 rows of w_attn per core
P_SHARD = H // NCORES  # 512 rows of w_proj per core

KC = H // 128  # 32 contraction chunks of the model dim
NB = S // 512  # 4 seq blocks of 512
MT = M_SHARD // 128  # 6 row tiles of qkv_t
ST = S // 128  # 16 seq tiles of 128
NEG = -1.0e9


def build_module() -> bass.Bass:
    nc = bacc.Bacc(
        "TRN2",
        target_bir_lowering=False,
        debug=False,
        num_devices=NCORES,
    )

    hs_t = nc.dram_tensor("hs_t", [H, S], BF16, kind="ExternalInput")
    wa_t = nc.dram_tensor("wa_t", [H, M_SHARD], BF16, kind="ExternalInput")
    wp_t = nc.dram_tensor("wp_t", [H, P_SHARD], BF16, kind="ExternalInput")
    cos_t = nc.dram_tensor("cos_t", [HD, S], BF16, kind="ExternalInput")
    sin_t = nc.dram_tensor("sin_t", [HD, S], BF16, kind="ExternalInput")
    rot_t = nc.dram_tensor("rot_t", [HD, HD], BF16, kind="ExternalInput")
    stair_in = nc.dram_tensor("stair_in", [128, 512], BF16, kind="ExternalInput")
    ones_in = nc.dram_tensor("ones_in", [128, 128], BF16, kind="ExternalInput")
    ident_in = nc.dram_tensor("ident_in", [128, 128], BF16, kind="ExternalInput")
    y_out = nc.dram_tensor("y", [S, P_SHARD], F32, kind="ExternalOutput")

    # per-seq-chunk collective buffers
    ag_ins = [
        nc.dram_tensor(f"ag_in{i}", [GROUP * HD, 512], BF16, kind="Internal")
        for i in range(NB)
    ]
    ag_outs = [
        nc.dram_tensor(
            f"ag_out{i}", [H, 512], BF16, kind="Internal", addr_space="Shared"
        )
        for i in range(NB)
    ]

    # DRAM views with 128-partition tiling of the contraction axis
    hs_v = hs_t[:].rearrange("(ko p) n -> p ko n", p=128)  # [128, 32, 2048]
    wa_v = wa_t[:].rearrange("(ko p) m -> p ko m", p=128)  # [128, 32, 768]
    wp_v = wp_t[:].rearrange("(ko p) m -> p ko m", p=128)  # [128, 32, 512]
    ag_vs = [a[:].rearrange("(ko p) n -> p ko n", p=128) for a in ag_outs]

    with tile.TileContext(nc) as tc:
        # ---------- persistent pools ----------
        const_pool = tc.alloc_tile_pool(name="consts", bufs=1)
        qkv_pool = tc.alloc_tile_pool(name="qkv", bufs=1)
        vnat_pool = tc.alloc_tile_pool(name="vnat", bufs=1)
        wp_pool = tc.alloc_tile_pool(name="wp", bufs=1)

        ones_sb = const_pool.tile([128, 128], BF16)
        ident_sb = const_pool.tile([128, 128], BF16)
        rot_sb = const_pool.tile([128, HD], BF16)
        stair_sb = const_pool.tile([128, 512], BF16)
        nc.scalar.dma_start(out=ones_sb, in_=ones_in[:])
        nc.scalar.dma_start(out=ident_sb, in_=ident_in[:])
        nc.scalar.dma_start(out=rot_sb, in_=rot_t[:])
        nc.scalar.dma_start(out=stair_sb, in_=stair_in[:])

        qkv_sb = qkv_pool.tile([128, MT, S], BF16)  # 24KB/part
        v_nat = vnat_pool.tile([128, ST, HD], BF16)  # 4KB/part
        wp_sb = wp_pool.tile([128, KC, P_SHARD], BF16)  # 32KB/part

        # ---------- phase A: qkv_t = wa_shard @ hs.T, rope fused ----------
        with (
            tc.tile_pool(name="wa", bufs=1) as wa_pool,
            tc.tile_pool(name="hs", bufs=2) as hs_pool,
            tc.tile_pool(name="rope", bufs=1) as rope_pool,
            tc.tile_pool(name="rs", bufs=2) as rs_pool,
            tc.tile_pool(name="psA", bufs=1, space="PSUM") as psA,
            tc.tile_pool(name="psR", bufs=2, space="PSUM") as psR,
        ):
            wa_sb = wa_pool.tile([128, KC, M_SHARD], BF16)  # 48KB/part
            cos_sb = rope_pool.tile([128, S], BF16, tag="cos")
            sin_sb = rope_pool.tile([128, S], BF16, tag="sin")

            # first wa chunk ASAP on scalar; cos/sin small on sync
            nc.scalar.dma_start(out=wa_sb[:, 0:8, :], in_=wa_v[:, 0:8, :])
            nc.sync.dma_start(out=cos_sb, in_=cos_t[:])
            nc.sync.dma_start(out=sin_sb, in_=sin_t[:])
            for kk in range(8, KC, 8):
                nc.scalar.dma_start(
                    out=wa_sb[:, kk : kk + 8, :], in_=wa_v[:, kk : kk + 8, :]
                )
            # w_proj stream behind wa on the same queue (needed much later)
            for kk in range(0, KC, 8):
                nc.scalar.dma_start(
                    out=wp_sb[:, kk : kk + 8, :], in_=wp_v[:, kk : kk + 8, :]
                )

            KSLAB = 8
            for nb in range(NB):
                blk = slice(nb * 512, (nb + 1) * 512)
                psums = [
                    psA.tile([128, 512], F32, tag=f"ps{m}", name=f"psA{m}")
                    for m in range(MT)
                ]
                for ks in range(0, KC, KSLAB):
                    hs_slab = hs_pool.tile([128, KSLAB, 512], BF16, name="hs_slab")
                    nc.sync.dma_start(out=hs_slab, in_=hs_v[:, ks : ks + KSLAB, blk])
                    for k in range(ks, ks + KSLAB):
                        for m in range(MT):
                            nc.tensor.matmul(
                                psums[m],
                                lhsT=wa_sb[:, k, m * 128 : (m + 1) * 128],
                                rhs=hs_slab[:, k - ks, :],
                                start=(k == 0),
                                stop=(k == KC - 1),
                            )
                for m in range(MT):
                    nc.vector.tensor_copy(out=qkv_sb[:, m, blk], in_=psums[m])

                # v natural layout via DMA-transpose (sync queue, tiny)
                for u in range(4):
                    nc.sync.dma_start_transpose(
                        v_nat[:, nb * 4 + u, :],
                        qkv_sb[:, GROUP + 1, nb * 512 + u * 128 : nb * 512 + (u + 1) * 128],
                    )

                # rope this seq block, k tile first (attention needs kT whole)
                for t in (GROUP, 0, 1, 2, 3):
                    x = qkv_sb[:, t, blk]
                    rp = psR.tile([128, 512], F32, name="rp")
                    nc.tensor.matmul(rp, lhsT=rot_sb[:], rhs=x, start=True, stop=True)
                    rs = rs_pool.tile([128, 512], BF16, name="rs")
                    nc.vector.tensor_mul(rs, rp, sin_sb[:, blk])
                    nc.vector.tensor_mul(x, x, cos_sb[:, blk])
                    nc.vector.tensor_add(x, x, rs)

        # ---------- phase B: attention + chunked AG + c_proj ----------
        with (
            tc.tile_pool(name="pt", bufs=1) as pt_pool,
            tc.tile_pool(name="attn", bufs=2) as attn_pool,
            tc.tile_pool(name="lh", bufs=4) as lh_pool,
            tc.tile_pool(name="ysb", bufs=2) as y_pool,
            tc.tile_pool(name="psS", bufs=3, space="PSUM") as psS,
            tc.tile_pool(name="psL", bufs=1, space="PSUM") as psL,
            tc.tile_pool(name="psO", bufs=2, space="PSUM") as psO,
            tc.tile_pool(name="psC", bufs=2, space="PSUM") as psC,
        ):
            # pre-zero the shifted-diagonal pt tags' dead zones once
            for r in (128, 256, 384):
                ptd = pt_pool.tile(
                    [128, 512], BF16, tag=f"ptd{r}", name=f"ptd{r}"
                )
                nc.vector.memset(ptd[:, 0:r], 0.0)

            kT = qkv_sb[:, GROUP, :]
            lh_pending = {}  # iq -> list of (sub, lh tile)

            def emit_lh(iq, sub):
                lh = lh_pool.tile([128, KC, 128], BF16, tag="lh", name="lh")
                nc.scalar.dma_start(
                    out=lh, in_=ag_vs[iq][:, :, sub * 128 : (sub + 1) * 128]
                )
                return lh

            def cproj(iq):
                tiles = lh_pending.pop(iq)
                for sub in range(4):
                    if sub >= len(tiles):
                        tiles.append(emit_lh(iq, sub))
                    lh = tiles[sub]
                    yp = psC.tile([128, 512], F32, name="yp")
                    for k in range(KC):
                        nc.tensor.matmul(
                            yp,
                            lhsT=lh[:, k, :],
                            rhs=wp_sb[:, k, :],
                            start=(k == 0),
                            stop=(k == KC - 1),
                        )
                    ysb = y_pool.tile([128, P_SHARD], F32, name="ysb")
                    nc.vector.tensor_copy(out=ysb, in_=yp)
                    nc.sync.dma_start(
                        out=y_out[(iq * 4 + sub) * 128 : (iq * 4 + sub + 1) * 128, :],
                        in_=ysb,
                    )

            order = [3, 2, 1, 0]
            for idx, iq in enumerate(order):
                njb = 4 * iq + 4
                q0 = iq * 512
                for h in range(GROUP):
                    l_ps = psL.tile([128, 512], F32, name="l_ps")
                    o_ps = psO.tile([128, 512], F32, name="o_ps")

                    # round r: key tile j=r; last 4 rounds are diagonal with
                    # shifted q-range [rel, 512) and a staircase bias matmul
                    sts = []  # per round: (st psum, pt tile, rel)

                    def emit_st(r):
                        j = r
                        rel = max(0, (j - 4 * iq) * 128)
                        st = psS.tile([128, 512], F32, name="st")
                        nc.tensor.matmul(
                            st[:, rel:512],
                            lhsT=kT[:, j * 128 : (j + 1) * 128],
                            rhs=qkv_sb[:, h, q0 + rel : q0 + 512],
                            start=True,
                            stop=(rel == 0),
                        )
                        if rel > 0:
                            nc.tensor.matmul(
                                st[:, rel:512],
                                lhsT=ident_sb[:],
                                rhs=stair_sb[:, 0 : 512 - rel],
                                start=False,
                                stop=True,
                            )
                        tag = f"ptd{rel}" if rel else "pt"
                        bufs = 1 if rel else 4
                        pt = pt_pool.tile(
                            [128, 512], BF16, tag=tag, bufs=bufs, name="pt"
                        )
                        nc.scalar.activation(
                            out=pt[:, rel:512],
                            in_=st[:, rel:512],
                            func=mybir.ActivationFunctionType.Exp,
                            scale=SCALE,
                        )
                        sts.append((st, pt, rel))

                    def emit_st_diag0(r):
                        # diagonal tile at rel==0 (j == 4*iq): triangle bias
                        j = r
                        st = psS.tile([128, 512], F32, name="st")
                        nc.tensor.matmul(
                            st,
                            lhsT=kT[:, j * 128 : (j + 1) * 128],
                            rhs=qkv_sb[:, h, q0 : q0 + 512],
                            start=True,
                            stop=False,
                        )
                        nc.tensor.matmul(
                            st,
                            lhsT=ident_sb[:],
                            rhs=stair_sb[:],
                            start=False,
                            stop=True,
                        )
                        pt = pt_pool.tile([128, 512], BF16, tag="pt", bufs=4, name="pt")
                        nc.scalar.activation(
                            out=pt,
                            in_=st,
                            func=mybir.ActivationFunctionType.Exp,
                            scale=SCALE,
                        )
                        sts.append((st, pt, 0))

                    def emit_round_st(r):
                        if r == 4 * iq:
                            emit_st_diag0(r)
                        else:
                            emit_st(r)

                    def emit_lo(r):
                        _, pt, _ = sts[r]
                        j = r
                        nc.tensor.matmul(
                            l_ps,
                            lhsT=ones_sb[:],
                            rhs=pt[:],
                            start=(r == 0),
                            stop=(r == njb - 1),
                        )
                        nc.tensor.matmul(
                            o_ps,
                            lhsT=v_nat[:, j, :],
                            rhs=pt[:],
                            start=(r == 0),
                            stop=(r == njb - 1),
                        )

                    depth = min(3, njb)
                    for r in range(depth):
                        emit_round_st(r)
                    for r in range(njb):
                        emit_lo(r)
                        if r + depth < njb:
                            emit_round_st(r + depth)

                    linv = attn_pool.tile([128, 512], F32, tag="linv", name="linv")
                    nc.vector.reciprocal_approx_fast(out=linv, in_=l_ps)
                    at = attn_pool.tile([128, 512], BF16, tag="at", name="at")
                    nc.vector.tensor_mul(at, o_ps, linv)
                    nc.sync.dma_start(
                        out=ag_ins[iq][h * 128 : (h + 1) * 128, :], in_=at
                    )

                # seq-chunked AllGather; overlaps the next chunk's compute
                nc.gpsimd.collective_compute(
                    "AllGather",
                    mybir.AluOpType.bypass,
                    replica_groups=[list(range(NCORES))],
                    ins=[ag_ins[iq][:]],
                    outs=[ag_outs[iq][:]],
                )

                if idx >= 1:
                    cproj(order[idx - 1])
                # prefetch first half of this chunk's c_proj activations
                # (after the interleaved cproj so its lh DMAs, which wait on
                # this chunk's AllGather, don't block the queue)
                lh_pending[iq] = [emit_lh(iq, 0), emit_lh(iq, 1)]
            cproj(order[-1])

        wp_pool.release()
        vnat_pool.release()
        qkv_pool.release()
        const_pool.release()

    nc.compile()
    return nc


_CACHED = {}


def _get_module():
    if "nc" not in _CACHED:
        _CACHED["nc"] = build_module()
    return _CACHED["nc"]


def make_in_maps(hidden_states, w_attn, w_proj, rope_cos, rope_sin):
    bf = ml_dtypes.bfloat16
    hidden_states = np.asarray(hidden_states, dtype=np.float32)
    w_attn = np.asarray(w_attn, dtype=np.float32)
    w_proj = np.asarray(w_proj, dtype=np.float32)
    rope_cos = np.asarray(rope_cos, dtype=np.float32)
    rope_sin = np.asarray(rope_sin, dtype=np.float32)

    hs_t = np.ascontiguousarray(hidden_states.reshape(S, H).T).astype(bf)
    cos_t = np.ascontiguousarray(rope_cos.T).astype(bf)
    sin_t = np.ascontiguousarray(rope_sin.T).astype(bf)

    # rotate-half as a matmul: rot(x) = R @ x for x in [HD, S] layout,
    # rot_t = R.T so that lhsT.T @ x = R @ x
    rot_t = np.zeros((HD, HD), dtype=np.float32)
    half = HD // 2
    rot_t[half + np.arange(half), np.arange(half)] = -1.0
    rot_t[np.arange(half), half + np.arange(half)] = 1.0

    # causal triangle bias: stair[k, u] = NEG iff u < k (else 0); a diag
    # key tile at relative offset rel uses cols [0, 512-rel)
    kk, uu = np.meshgrid(np.arange(128), np.arange(512), indexing="ij")
    stair = np.where(uu < kk, NEG, 0.0).astype(np.float32)

    ones = np.ones((128, 128), dtype=np.float32)
    ident = np.eye(128, dtype=np.float32)

    in_maps = []
    for i in range(NCORES):
        wa_sh = w_attn[i * M_SHARD : (i + 1) * M_SHARD, :]
        wp_sh = w_proj[i * P_SHARD : (i + 1) * P_SHARD, :]
        in_maps.append(
            {
                "hs_t": hs_t,
                "wa_t": np.ascontiguousarray(wa_sh.T).astype(bf),
                "wp_t": np.ascontiguousarray(wp_sh.T).astype(bf),
                "cos_t": cos_t,
                "sin_t": sin_t,
                "rot_t": rot_t.astype(bf),
                "stair_in": stair.astype(bf),
                "ones_in": ones.astype(bf),
                "ident_in": ident.astype(bf),
            }
        )
    return in_maps


def kernel(hidden_states, w_attn, w_proj, rope_cos, rope_sin, **_unused):
    nc = _get_module()
    in_maps = make_in_maps(hidden_states, w_attn, w_proj, rope_cos, rope_sin)
    res = run_bass_kernel_spmd(nc, in_maps, core_ids=list(range(NCORES)))

    out = np.empty((S, H), dtype=np.float32)
    for i in range(NCORES):
        out[:, i * P_SHARD : (i + 1) * P_SHARD] = res.results[i]["y"]
    return out.reshape(B, S, H)
